# revision 17
# baseline (speedup 1.0000x reference)
"""Fused fake-quant GEMM + bias + residual + LayerNorm (BertSelfOutput) on 8 trn2 cores.

Strategy: data-parallel over the batch dim (B=8 -> one batch element per core).
Each core computes, for its [4096, 1024] shard:
    hq = fake_quant(hidden); wq = fake_quant(weight)
    h  = hq @ wq.T + bias;   y = h + input;   out = layernorm(y) * gamma + beta

Key tricks:
- fake-quant values are integers in [-127, 127] after scaling; exactly
  representable in bf16 -> GEMM runs at full PE bf16 rate with exact fp32
  integer accumulation in PSUM; one dequant multiply at the end matches the
  fp32 reference to ~2e-6 relative.
- hidden/weight are pre-transposed on the host (layout-only prep) so the
  contraction dim lands on partitions with zero on-chip transposes.
- rounding = clamp(x*s, +-127) then +/- 1.5*2^23 on DVE: exact IEEE
  round-half-to-even, bit-identical to jnp.round.
- bias rides as a K=2 matmul row pair (bf16 hi+lo split, exact to ~1e-7).
- LN mean comes free from the dequant+residual pass's accum_out; sum(y^2)
  from an ACT Square accum_out; per-group batched stat math; the final
  (y-mu)*rs affine runs on ACT with per-partition scale/bias.
"""

import numpy as np
import ml_dtypes

import concourse.bass as bass
import concourse.mybir as mybir
import concourse.tile as tile
from concourse import bacc
from concourse.bass_utils import run_bass_kernel_spmd

F32 = mybir.dt.float32
BF16 = mybir.dt.bfloat16
AF = mybir.ActivationFunctionType
OP = mybir.AluOpType

MAGIC = 12582912.0  # 1.5 * 2**23: (x + MAGIC) - MAGIC == rint(x) for |x| < 2**22
QMAX = 127.0
CLIP_VAL = 2.5
LN_EPS = 1e-12
H = 1024
N_CORES = 8
P = 128
G = 8  # m-tiles per stats group (one super-block)


def _scale_sym(x: np.ndarray) -> np.float32:
    """fp32-exact replica of the reference's per-tensor scale computation."""
    amax = np.float32(min(np.float32(np.abs(x).max()), np.float32(CLIP_VAL)))
    return np.float32(np.float32(QMAX) / np.maximum(amax, np.float32(1e-8)))


def _quant3(nc, pool, src, s, tag, out_ap):
    """out_ap = bf16(round_half_even(clamp(src*s, +-127))), all exact IEEE fp32."""
    n = src.shape[-1]
    a = pool.tile([P, n], F32, tag=f"{tag}a")
    nc.vector.tensor_scalar(
        out=a, in0=src, scalar1=float(s), scalar2=QMAX, op0=OP.mult, op1=OP.min
    )
    b = pool.tile([P, n], F32, tag=f"{tag}b")
    nc.vector.tensor_scalar(
        out=b, in0=a, scalar1=-QMAX, scalar2=MAGIC, op0=OP.max, op1=OP.add
    )
    nc.vector.tensor_scalar(
        out=out_ap, in0=b, scalar1=MAGIC, scalar2=None, op0=OP.subtract
    )


def build_bass(n_rows: int, s_h: float, s_w: float, deq: float, trivial_ln: bool):
    nc = bacc.Bacc(num_devices=N_CORES)
    KT = H // P  # 8 k-tiles
    SB = n_rows // (P * G)  # super-blocks (each G m-tiles)
    assert SB * P * G == n_rows

    hst = nc.declare_dram_parameter("hst", [H, n_rows], F32, isOutput=False)  # hidden.T
    res = nc.declare_dram_parameter("res", [n_rows, H], F32, isOutput=False)
    wt = nc.declare_dram_parameter("wt", [H, H], F32, isOutput=False)  # weight.T
    biasq = nc.declare_dram_parameter("biasq", [2, H], BF16, isOutput=False)
    ones2 = nc.declare_dram_parameter("ones2", [2, P], BF16, isOutput=False)
    if not trivial_ln:
        gamma = nc.declare_dram_parameter("gamma", [H], F32, isOutput=False)
        beta = nc.declare_dram_parameter("beta", [H], F32, isOutput=False)
    out = nc.declare_dram_parameter("out", [n_rows, H], F32, isOutput=True)

    with tile.TileContext(nc) as tc:
        with (
            tc.tile_pool(name="singles", bufs=1) as singles,
            tc.tile_pool(name="wprep", bufs=1) as wprep,
            tc.tile_pool(name="hin", bufs=6) as hin,
            tc.tile_pool(name="quant", bufs=2) as quant,
            tc.tile_pool(name="qkeep", bufs=2 * KT + 1) as qkeep,
            tc.tile_pool(name="resin", bufs=6) as resin,
            tc.tile_pool(name="ystore", bufs=G + 2) as ystore,
            tc.tile_pool(name="oout", bufs=4) as oout,
            tc.tile_pool(name="stat", bufs=2) as stat,
            tc.tile_pool(name="pso", bufs=3, space="PSUM") as pso_pool,
            tc.tile_pool(name="psq", bufs=1, space="PSUM") as psq_pool,
        ):
            # ---- constants
            ones_t = singles.tile([2, P], BF16)
            nc.sync.dma_start(out=ones_t, in_=ones2[:, :])
            biasq_t = singles.tile([2, H], BF16)
            nc.sync.dma_start(out=biasq_t, in_=biasq[:, :])
            eps_t = singles.tile([P, 1], F32)
            nc.vector.memset(eps_t, LN_EPS)
            if not trivial_ln:
                gamma_t = singles.tile([P, H], F32)
                nc.sync.dma_start(
                    out=gamma_t,
                    in_=bass.AP(tensor=gamma.tensor, offset=0, ap=[[0, P], [1, H]]),
                )
                beta_t = singles.tile([P, H], F32)
                nc.sync.dma_start(
                    out=beta_t,
                    in_=bass.AP(tensor=beta.tensor, offset=0, ap=[[0, P], [1, H]]),
                )

            # ---- weight: already [h, o] on host; quantize in place (no transpose)
            wqt = singles.tile([P, KT, H], BF16)
            for k in range(KT):
                wtile = wprep.tile([P, H], F32, tag="wt")
                nc.sync.dma_start(out=wtile, in_=wt[k * P : (k + 1) * P, :])
                _quant3(nc, wprep, wtile, s_w, "w", wqt[:, k, :])

            # ---- main loop over super-blocks of G m-tiles
            # The quantize for super-block s+1 is interleaved into the m-tile
            # loop of super-block s (one k-tile per m-tile; G == KT) so the
            # in-order DVE never stalls the PE between super-blocks.
            def quant_ktile(s, k):
                mcols = slice(s * P * G, (s + 1) * P * G)
                htile = hin.tile([P, P * G], F32)
                nc.sync.dma_start(out=htile, in_=hst[k * P : (k + 1) * P, mcols])
                qs = qkeep.tile([P, P * G], BF16)
                _quant3(nc, quant, htile, s_h, "h", qs)
                return qs

            qk = [quant_ktile(0, k) for k in range(KT)]
            for s in range(SB):
                qk_next = []
                meansum = stat.tile([P, G], F32, tag="msum")
                sqsum = stat.tile([P, G], F32, tag="sqsum")
                ys = []
                for mt in range(G):
                    mrow = slice((s * G + mt) * P, (s * G + mt + 1) * P)
                    pso = pso_pool.tile([P, H], F32, tag="pso")
                    for nh in range(2):
                        col = slice(nh * 512, (nh + 1) * 512)
                        for k in range(KT):
                            nc.tensor.matmul(
                                pso[:, col],
                                lhsT=qk[k][:, mt * P : (mt + 1) * P],
                                rhs=wqt[:, k, col],
                                start=(k == 0),
                                stop=False,
                            )
                        nc.tensor.matmul(
                            pso[:, col],
                            lhsT=ones_t[:, :],
                            rhs=biasq_t[:, col],
                            start=False,
                            stop=True,
                        )
                    rt = resin.tile([P, H], F32)
                    nc.sync.dma_start(out=rt, in_=res[mrow, :])
                    # y = pso * deq + input; row-sums accumulate for the mean
                    yt = ystore.tile([P, H], F32, tag="y")
                    nc.vector.scalar_tensor_tensor(
                        out=yt,
                        in0=pso,
                        scalar=float(deq),
                        in1=rt,
                        op0=OP.mult,
                        op1=OP.add,
                        accum_out=meansum[:, mt : mt + 1],
                    )
                    # sum(y^2) via ACT Square accumulate (scratch result in PSUM)
                    sq = psq_pool.tile([P, H], F32)
                    nc.scalar.activation(
                        sq, yt, AF.Square, accum_out=sqsum[:, mt : mt + 1]
                    )
                    ys.append(yt)
                    # pipelined quantize of the next super-block's k-tile mt
                    if s + 1 < SB:
                        qk_next.append(quant_ktile(s + 1, mt))

                # group stats: mu, var, rs = 1/sqrt(var+eps), shift = -mu*rs
                mu = stat.tile([P, G], F32, tag="mu")
                nc.vector.tensor_scalar(
                    out=mu, in0=meansum, scalar1=1.0 / H, scalar2=None, op0=OP.mult
                )
                mu2 = stat.tile([P, G], F32, tag="mu2")
                nc.vector.tensor_tensor(out=mu2, in0=mu, in1=mu, op=OP.mult)
                var = stat.tile([P, G], F32, tag="var")
                nc.vector.scalar_tensor_tensor(
                    out=var, in0=sqsum, scalar=1.0 / H, in1=mu2, op0=OP.mult, op1=OP.subtract
                )
                rs = stat.tile([P, G], F32, tag="rs")
                nc.scalar.activation(rs, var, AF.Sqrt, bias=eps_t[:, :], scale=1.0)
                nc.vector.reciprocal(out=rs, in_=rs)
                shift = stat.tile([P, G], F32, tag="shift")
                nc.vector.scalar_tensor_tensor(
                    out=shift, in0=mu, scalar=-1.0, in1=rs, op0=OP.mult, op1=OP.mult
                )

                for mt in range(G):
                    mrow = slice((s * G + mt) * P, (s * G + mt + 1) * P)
                    ot = oout.tile([P, H], F32)
                    nc.scalar.activation(
                        ot,
                        ys[mt],
                        AF.Identity,
                        bias=shift[:, mt : mt + 1],
                        scale=rs[:, mt : mt + 1],
                    )
                    if not trivial_ln:
                        nc.vector.tensor_mul(out=ot, in0=ot, in1=gamma_t)
                        nc.vector.tensor_add(out=ot, in0=ot, in1=beta_t)
                    nc.gpsimd.dma_start(out=out[mrow, :], in_=ot)
                qk = qk_next

    nc.compile()
    return nc


def _prepare(hidden_states, input_tensor, weight, bias, ln_gamma, ln_beta):
    B, S, Hdim = hidden_states.shape
    assert Hdim == H and B == N_CORES
    s_h = _scale_sym(hidden_states)
    s_w = _scale_sym(weight)
    deq = np.float32(1.0 / (np.float64(s_h) * np.float64(s_w)))

    bscaled = bias.astype(np.float64) * np.float64(s_h) * np.float64(s_w)
    b_hi = bscaled.astype(ml_dtypes.bfloat16)
    b_lo = (bscaled - b_hi.astype(np.float64)).astype(ml_dtypes.bfloat16)
    biasq = np.stack([b_hi, b_lo])  # [2, H] bf16

    trivial_ln = bool(np.all(ln_gamma == 1.0) and np.all(ln_beta == 0.0))

    ones2 = np.ones((2, P), dtype=ml_dtypes.bfloat16)
    common = {
        "wt": np.ascontiguousarray(weight.T),
        "biasq": biasq,
        "ones2": ones2,
    }
    if not trivial_ln:
        common["gamma"] = np.ascontiguousarray(ln_gamma, dtype=np.float32)
        common["beta"] = np.ascontiguousarray(ln_beta, dtype=np.float32)

    in_maps = []
    for b in range(N_CORES):
        in_maps.append(
            {
                "hst": np.ascontiguousarray(hidden_states[b].T),
                "res": np.ascontiguousarray(input_tensor[b]),
                **common,
            }
        )
    return s_h, s_w, deq, trivial_ln, in_maps, S


def _ensure_ntff_hook():
    """Provide antenv.axon_hooks if the image lacks it (NTFF tracing)."""
    import sys
    import types

    try:
        from antenv.axon_hooks import get_axon_ntff_profile_hook  # noqa: F401

        return
    except ImportError:
        pass
    from trn_agent_boot.trn_boot import _ntff_profile_via_ctypes

    hook = _ntff_profile_via_ctypes("/opt/axon/libaxon_pjrt.so")
    mod = types.ModuleType("antenv.axon_hooks")
    mod.get_axon_ntff_profile_hook = lambda: hook
    mod.set_axon_ntff_profile_hook = lambda h: None
    sys.modules["antenv.axon_hooks"] = mod


def run(hidden_states, input_tensor, weight, bias, ln_gamma, ln_beta, trace=False, **trace_kw):
    if trace:
        _ensure_ntff_hook()
    s_h, s_w, deq, trivial_ln, in_maps, S = _prepare(
        hidden_states, input_tensor, weight, bias, ln_gamma, ln_beta
    )
    nc = build_bass(S, s_h, s_w, deq, trivial_ln)
    kres = run_bass_kernel_spmd(nc, in_maps, list(range(N_CORES)), trace=trace, **trace_kw)
    out = np.stack([kres.results[i]["out"] for i in range(N_CORES)])
    return out, kres


def kernel(hidden_states, input_tensor, weight, bias, ln_gamma, ln_beta):
    out, _ = run(hidden_states, input_tensor, weight, bias, ln_gamma, ln_beta)
    return out


# revision 20
# speedup vs baseline: 1.1629x; 1.1629x over previous
"""Fused fake-quant GEMM + bias + residual + LayerNorm (BertSelfOutput) on 8 trn2 cores.

Strategy: data-parallel over the batch dim (B=8 -> one batch element per core).
Each core computes, for its [4096, 1024] shard:
    hq = fake_quant(hidden); wq = fake_quant(weight)
    h  = hq @ wq.T + bias;   y = h + input;   out = layernorm(y) * gamma + beta

Key tricks:
- fake-quant values are integers in [-127, 127] after scaling; exactly
  representable in bf16 -> GEMM runs at full PE bf16 rate with exact fp32
  integer accumulation in PSUM; one dequant multiply at the end matches the
  fp32 reference to ~2e-6 relative.
- hidden/weight are pre-transposed on the host (layout-only prep) so the
  contraction dim lands on partitions with zero on-chip transposes.
- rounding = clamp(x*s, +-127) then +/- 1.5*2^23 on DVE: exact IEEE
  round-half-to-even, bit-identical to jnp.round.
- bias rides as a K=2 matmul row pair (bf16 hi+lo split, exact to ~1e-7).
- LN mean comes free from the dequant+residual pass's accum_out; sum(y^2)
  from an ACT Square accum_out; per-group batched stat math; the final
  (y-mu)*rs affine runs on ACT with per-partition scale/bias.
"""

import numpy as np
import ml_dtypes

import concourse.bass as bass
import concourse.mybir as mybir
import concourse.tile as tile
from concourse import bacc
from concourse.bass_utils import run_bass_kernel_spmd

F32 = mybir.dt.float32
BF16 = mybir.dt.bfloat16
AF = mybir.ActivationFunctionType
OP = mybir.AluOpType

MAGIC = 12582912.0  # 1.5 * 2**23: (x + MAGIC) - MAGIC == rint(x) for |x| < 2**22
QMAX = 127.0
CLIP_VAL = 2.5
LN_EPS = 1e-12
H = 1024
N_CORES = 8
P = 128
G = 8  # m-tiles per stats group (one super-block)


def _scale_sym(x: np.ndarray) -> np.float32:
    """fp32-exact replica of the reference's per-tensor scale computation."""
    amax = np.float32(min(np.float32(np.abs(x).max()), np.float32(CLIP_VAL)))
    return np.float32(np.float32(QMAX) / np.maximum(amax, np.float32(1e-8)))


def _quant3(nc, pool, src, s, tag, out_ap):
    """out_ap = bf16(round_half_even(clamp(src*s, +-127))), all exact IEEE fp32."""
    n = src.shape[-1]
    a = pool.tile([P, n], F32, tag=f"{tag}a")
    nc.vector.tensor_scalar(
        out=a, in0=src, scalar1=float(s), scalar2=QMAX, op0=OP.mult, op1=OP.min
    )
    b = pool.tile([P, n], F32, tag=f"{tag}b")
    nc.vector.tensor_scalar(
        out=b, in0=a, scalar1=-QMAX, scalar2=MAGIC, op0=OP.max, op1=OP.add
    )
    nc.vector.tensor_scalar(
        out=out_ap, in0=b, scalar1=MAGIC, scalar2=None, op0=OP.subtract
    )


def build_bass(n_rows: int, s_h: float, s_w: float, deq: float, trivial_ln: bool):
    nc = bacc.Bacc(num_devices=N_CORES)
    KT = H // P  # 8 k-tiles
    SB = n_rows // (P * G)  # super-blocks (each G m-tiles)
    assert SB * P * G == n_rows

    hst = nc.declare_dram_parameter("hst", [H, n_rows], F32, isOutput=False)  # hidden.T
    res = nc.declare_dram_parameter("res", [n_rows, H], F32, isOutput=False)
    wt = nc.declare_dram_parameter("wt", [H, H], F32, isOutput=False)  # weight.T
    biasq = nc.declare_dram_parameter("biasq", [2, H], BF16, isOutput=False)
    ones2 = nc.declare_dram_parameter("ones2", [2, P], BF16, isOutput=False)
    if not trivial_ln:
        gamma = nc.declare_dram_parameter("gamma", [H], F32, isOutput=False)
        beta = nc.declare_dram_parameter("beta", [H], F32, isOutput=False)
    out = nc.declare_dram_parameter("out", [n_rows, H], F32, isOutput=True)

    with tile.TileContext(nc) as tc:
        with (
            tc.tile_pool(name="singles", bufs=1) as singles,
            tc.tile_pool(name="wprep", bufs=1) as wprep,
            tc.tile_pool(name="hin", bufs=6) as hin,
            tc.tile_pool(name="quant", bufs=2) as quant,
            tc.tile_pool(name="qkeep", bufs=2 * KT + 1) as qkeep,
            tc.tile_pool(name="resin", bufs=6) as resin,
            tc.tile_pool(name="ystore", bufs=G + 2) as ystore,
            tc.tile_pool(name="oout", bufs=4) as oout,
            tc.tile_pool(name="stat", bufs=2) as stat,
            tc.tile_pool(name="pso", bufs=4, space="PSUM") as pso_pool,
            tc.tile_pool(name="sqscr", bufs=1) as psq_pool,
        ):
            # ---- constants
            ones_t = singles.tile([2, P], BF16)
            nc.sync.dma_start(out=ones_t, in_=ones2[:, :])
            biasq_t = singles.tile([2, H], BF16)
            nc.sync.dma_start(out=biasq_t, in_=biasq[:, :])
            eps_t = singles.tile([P, 1], F32)
            nc.vector.memset(eps_t, LN_EPS)
            if not trivial_ln:
                gamma_t = singles.tile([P, H], F32)
                nc.sync.dma_start(
                    out=gamma_t,
                    in_=bass.AP(tensor=gamma.tensor, offset=0, ap=[[0, P], [1, H]]),
                )
                beta_t = singles.tile([P, H], F32)
                nc.sync.dma_start(
                    out=beta_t,
                    in_=bass.AP(tensor=beta.tensor, offset=0, ap=[[0, P], [1, H]]),
                )

            # ---- weight quant (host-pretransposed) interleaved with the first
            # super-block's hidden quant so matmuls can start after k=0 is ready
            def quant_ktile(s, k):
                mcols = slice(s * P * G, (s + 1) * P * G)
                htile = hin.tile([P, P * G], F32)
                nc.sync.dma_start(out=htile, in_=hst[k * P : (k + 1) * P, mcols])
                qs = qkeep.tile([P, P * G], BF16)
                _quant3(nc, quant, htile, s_h, "h", qs)
                return qs

            wqt = singles.tile([P, KT, H], BF16)
            qk = []
            res_pref = []
            for k in range(KT):
                wtile = wprep.tile([P, H], F32, tag="wt")
                nc.sync.dma_start(out=wtile, in_=wt[k * P : (k + 1) * P, :])
                # weight quant: scale on ACT (idle at startup), round+clamp on DVE
                uw = wprep.tile([P, H], F32, tag="uw")
                nc.scalar.activation(uw, wtile, AF.Copy, bias=0.0, scale=float(s_w))
                rw = wprep.tile([P, H], BF16, tag="rw")
                nc.vector.tensor_scalar(
                    out=rw, in0=uw, scalar1=MAGIC, scalar2=MAGIC, op0=OP.add, op1=OP.subtract
                )
                nc.vector.tensor_scalar(
                    out=wqt[:, k, :], in0=rw, scalar1=QMAX, scalar2=-QMAX, op0=OP.min, op1=OP.max
                )
                qk.append(quant_ktile(0, k))
                if k < 4:  # early residual prefetch so stt(0..3) aren't starved
                    rt0 = resin.tile([P, H], F32, tag="rt")
                    nc.sync.dma_start(out=rt0, in_=res[k * P : (k + 1) * P, :])
                    res_pref.append(rt0)
            for s in range(SB):
                qk_next = []
                meansum = stat.tile([P, G], F32, tag="msum")
                sqsum = stat.tile([P, G], F32, tag="sqsum")
                ys = []
                for mt in range(G):
                    mrow = slice((s * G + mt) * P, (s * G + mt + 1) * P)
                    pso = pso_pool.tile([P, H], F32, tag="pso")
                    for nh in range(2):
                        col = slice(nh * 512, (nh + 1) * 512)
                        for k in range(KT):
                            nc.tensor.matmul(
                                pso[:, col],
                                lhsT=qk[k][:, mt * P : (mt + 1) * P],
                                rhs=wqt[:, k, col],
                                start=(k == 0),
                                stop=False,
                            )
                        nc.tensor.matmul(
                            pso[:, col],
                            lhsT=ones_t[:, :],
                            rhs=biasq_t[:, col],
                            start=False,
                            stop=True,
                        )
                    if s == 0 and mt < 4:
                        rt = res_pref[mt]
                    else:
                        rt = resin.tile([P, H], F32, tag="rt")
                        nc.sync.dma_start(out=rt, in_=res[mrow, :])
                    # y = pso * deq + input; row-sums accumulate for the mean
                    yt = ystore.tile([P, H], F32, tag="y")
                    nc.vector.scalar_tensor_tensor(
                        out=yt,
                        in0=pso,
                        scalar=float(deq),
                        in1=rt,
                        op0=OP.mult,
                        op1=OP.add,
                        accum_out=meansum[:, mt : mt + 1],
                    )
                    # sum(y^2) via ACT Square accumulate (scratch result in PSUM)
                    sq = psq_pool.tile([P, H], F32)
                    nc.scalar.activation(
                        sq, yt, AF.Square, accum_out=sqsum[:, mt : mt + 1]
                    )
                    ys.append(yt)
                    # pipelined quantize of the next super-block's k-tile mt
                    if s + 1 < SB:
                        qk_next.append(quant_ktile(s + 1, mt))

                # group stats: mu, var, rs = 1/sqrt(var+eps), shift = -mu*rs
                mu = stat.tile([P, G], F32, tag="mu")
                nc.vector.tensor_scalar(
                    out=mu, in0=meansum, scalar1=1.0 / H, scalar2=None, op0=OP.mult
                )
                mu2 = stat.tile([P, G], F32, tag="mu2")
                nc.vector.tensor_tensor(out=mu2, in0=mu, in1=mu, op=OP.mult)
                var = stat.tile([P, G], F32, tag="var")
                nc.vector.scalar_tensor_tensor(
                    out=var, in0=sqsum, scalar=1.0 / H, in1=mu2, op0=OP.mult, op1=OP.subtract
                )
                rs = stat.tile([P, G], F32, tag="rs")
                nc.scalar.activation(rs, var, AF.Sqrt, bias=eps_t[:, :], scale=1.0)
                nc.vector.reciprocal(out=rs, in_=rs)
                shift = stat.tile([P, G], F32, tag="shift")
                nc.vector.scalar_tensor_tensor(
                    out=shift, in0=mu, scalar=-1.0, in1=rs, op0=OP.mult, op1=OP.mult
                )

                for mt in range(G):
                    mrow = slice((s * G + mt) * P, (s * G + mt + 1) * P)
                    ot = oout.tile([P, H], F32)
                    nc.scalar.activation(
                        ot,
                        ys[mt],
                        AF.Identity,
                        bias=shift[:, mt : mt + 1],
                        scale=rs[:, mt : mt + 1],
                    )
                    if not trivial_ln:
                        nc.vector.tensor_mul(out=ot, in0=ot, in1=gamma_t)
                        nc.vector.tensor_add(out=ot, in0=ot, in1=beta_t)
                    nc.gpsimd.dma_start(out=out[mrow, :], in_=ot)
                qk = qk_next

    nc.compile()
    return nc


def _prepare(hidden_states, input_tensor, weight, bias, ln_gamma, ln_beta):
    B, S, Hdim = hidden_states.shape
    assert Hdim == H and B == N_CORES
    s_h = _scale_sym(hidden_states)
    s_w = _scale_sym(weight)
    deq = np.float32(1.0 / (np.float64(s_h) * np.float64(s_w)))

    bscaled = bias.astype(np.float64) * np.float64(s_h) * np.float64(s_w)
    b_hi = bscaled.astype(ml_dtypes.bfloat16)
    b_lo = (bscaled - b_hi.astype(np.float64)).astype(ml_dtypes.bfloat16)
    biasq = np.stack([b_hi, b_lo])  # [2, H] bf16

    trivial_ln = bool(np.all(ln_gamma == 1.0) and np.all(ln_beta == 0.0))

    ones2 = np.ones((2, P), dtype=ml_dtypes.bfloat16)
    common = {
        "wt": np.ascontiguousarray(weight.T),
        "biasq": biasq,
        "ones2": ones2,
    }
    if not trivial_ln:
        common["gamma"] = np.ascontiguousarray(ln_gamma, dtype=np.float32)
        common["beta"] = np.ascontiguousarray(ln_beta, dtype=np.float32)

    in_maps = []
    for b in range(N_CORES):
        in_maps.append(
            {
                "hst": np.ascontiguousarray(hidden_states[b].T),
                "res": np.ascontiguousarray(input_tensor[b]),
                **common,
            }
        )
    return s_h, s_w, deq, trivial_ln, in_maps, S


def _ensure_ntff_hook():
    """Provide antenv.axon_hooks if the image lacks it (NTFF tracing)."""
    import sys
    import types

    try:
        from antenv.axon_hooks import get_axon_ntff_profile_hook  # noqa: F401

        return
    except ImportError:
        pass
    from trn_agent_boot.trn_boot import _ntff_profile_via_ctypes

    hook = _ntff_profile_via_ctypes("/opt/axon/libaxon_pjrt.so")
    mod = types.ModuleType("antenv.axon_hooks")
    mod.get_axon_ntff_profile_hook = lambda: hook
    mod.set_axon_ntff_profile_hook = lambda h: None
    sys.modules["antenv.axon_hooks"] = mod


def run(hidden_states, input_tensor, weight, bias, ln_gamma, ln_beta, trace=False, **trace_kw):
    if trace:
        _ensure_ntff_hook()
    s_h, s_w, deq, trivial_ln, in_maps, S = _prepare(
        hidden_states, input_tensor, weight, bias, ln_gamma, ln_beta
    )
    nc = build_bass(S, s_h, s_w, deq, trivial_ln)
    kres = run_bass_kernel_spmd(nc, in_maps, list(range(N_CORES)), trace=trace, **trace_kw)
    out = np.stack([kres.results[i]["out"] for i in range(N_CORES)])
    return out, kres


def kernel(hidden_states, input_tensor, weight, bias, ln_gamma, ln_beta):
    out, _ = run(hidden_states, input_tensor, weight, bias, ln_gamma, ln_beta)
    return out
